# revision 3
# baseline (speedup 1.0000x reference)
"""Distributed Bass kernel for nn_Attention (dense transformer attention block).

Tensor-parallel over heads across 8 TRN2 NeuronCores:
  - each core owns 4 heads: its slice of W_pack (QKV) and the matching
    512 input channels of o_proj,
  - attention (RoPE + causal softmax) is computed fully locally per head,
  - attention outputs are AllGathered (bf16) and each core computes a
    distinct 512-column slice of the o_proj output; the host concatenates
    the slices (no AllReduce needed).

Compute dtype: bf16 matmuls with f32 PSUM accumulation, f32 softmax.
"""

import sys
import types
import math

sys.path.insert(0, "/opt/trn_rl_repo")

import numpy as np
import ml_dtypes

from concourse import bacc, tile, mybir
from concourse.bass_utils import run_bass_kernel_spmd

BF16 = mybir.dt.bfloat16
F32 = mybir.dt.float32

B = 2
S = 2048
H = 4096
NH = 32
D = 128
T = B * S
N_CORES = 8
HEADS_PER_CORE = NH // N_CORES          # 4
CH = HEADS_PER_CORE * D                 # 512 channels per core
BASE = 10000.0
NEG = np.float32(np.finfo(np.float32).min)

# mask-block ops (per [q-chunk=512, k-tile=128] block, scoresT layout)
SKIP, NOMASK, DIAG, DMAMASK = 0, 1, 2, 3

QC = S // 512                            # 4 q-chunks per batch
KT = S // 128                            # 16 k-tiles per batch

_cache = {}
last_run_info = {}


def _ensure_trace_hook():
    """Register the NTFF profile hook missing from this image's antenv."""
    if "antenv.axon_hooks" in sys.modules:
        return
    try:
        from trn_agent_boot.trn_boot import _ntff_profile_via_ctypes

        hook = _ntff_profile_via_ctypes("/opt/axon/libaxon_pjrt.so")
        mod = types.ModuleType("antenv.axon_hooks")
        mod.get_axon_ntff_profile_hook = lambda: hook
        mod.set_axon_ntff_profile_hook = lambda h: None
        sys.modules["antenv.axon_hooks"] = mod
        from concourse import bass_utils

        bass_utils.upload_artifacts = lambda tmpdir: tmpdir
    except Exception:
        pass


def _classify_mask(mask):
    """Per (b, q-chunk 512, k-tile 128) block op for the scoresT layout."""
    ops = np.empty((B, QC, KT), dtype=np.int32)
    karr = np.arange(128)
    qarr = np.arange(512)
    need_dma = False
    for b in range(B):
        mb = np.asarray(mask[b, 0])
        for qc in range(QC):
            qs = qc * 512
            for kt in range(KT):
                ks = kt * 128
                sub = mb[qs : qs + 512, ks : ks + 128]  # [q, k]
                if np.all(sub <= -1e30):
                    ops[b, qc, kt] = SKIP
                elif not sub.any():
                    ops[b, qc, kt] = NOMASK
                else:
                    delta = ks - qs
                    if 0 <= delta <= 384:
                        pat = np.where(
                            (delta + karr[None, :]) > qarr[:, None], NEG, np.float32(0)
                        )
                        if np.array_equal(sub, pat):
                            ops[b, qc, kt] = DIAG
                            continue
                    ops[b, qc, kt] = DMAMASK
                    need_dma = True
    # every q row needs at least one unmasked block (else softmax sum = 0)
    return ops, need_dma


def _build(ops, need_dma):
    nc = bacc.Bacc(None, target_bir_lowering=False)

    x_t = nc.declare_dram_parameter("x_t", [H, T], BF16, isOutput=False)
    wqk = nc.declare_dram_parameter("wqk", [H, 2 * CH], BF16, isOutput=False)
    wv = nc.declare_dram_parameter("wv", [H, CH], BF16, isOutput=False)
    wo = nc.declare_dram_parameter("wo", [H, CH], BF16, isOutput=False)
    tabs = nc.declare_dram_parameter("tabs", [4, D, T], F32, isOutput=False)
    maskT = None
    if need_dma:
        maskT = nc.declare_dram_parameter("maskT", [B, S, S], F32, isOutput=False)
    out = nc.declare_dram_parameter("out", [T, CH], F32, isOutput=True)

    ones_col_np = np.ones((D, 1), dtype=ml_dtypes.bfloat16)
    ones_row_np = np.ones((1, D), dtype=np.float32)
    # maskfull[k, j] = NEG if k > j - 384 else 0   (slice [384-delta : 896-delta])
    j = np.arange(896)
    maskfull_np = np.where(np.arange(D)[:, None] > (j[None, :] - 384), NEG, 0.0).astype(
        np.float32
    )

    rg = [list(range(N_CORES))]

    with tile.TileContext(nc) as tc:
        with (
            tc.tile_pool(name="dram", bufs=1, space="DRAM") as dram,
            tc.tile_pool(name="const", bufs=1) as constp,
            tc.tile_pool(name="wpool", bufs=1) as wpool,
        ):
            qt_d = dram.tile([CH, T], BF16, tag="qt_d")
            kt_d = dram.tile([CH, T], BF16, tag="kt_d")
            v_d = dram.tile([T, CH], BF16, tag="v_d")
            att_loc = [dram.tile([CH, S], BF16, tag=f"att_loc{b}", name=f"att_loc{b}") for b in range(B)]
            att_all = [
                dram.tile([N_CORES * CH, S], BF16, addr_space="Shared", tag=f"att_all{b}", name=f"att_all{b}")
                for b in range(B)
            ]

            ones_col = constp.tile([D, 1], BF16, tag="ones_col")
            nc.sync.dma_start(ones_col[:], nc.inline_tensor(ones_col_np, "ones_col_c")[:])
            ones_row = constp.tile([1, D], F32, tag="ones_row")
            nc.sync.dma_start(ones_row[:], nc.inline_tensor(ones_row_np, "ones_row_c")[:])
            maskfull = constp.tile([D, 896], F32, tag="maskfull")
            nc.sync.dma_start(maskfull[:], nc.inline_tensor(maskfull_np, "maskfull_c")[:])

            # ---------------- stage 1a: Q,K projection + RoPE ----------------
            wqk_sb = wpool.tile([D, H // D, 2 * CH], BF16, tag="w")
            nc.sync.dma_start(wqk_sb[:], wqk.rearrange("(ho p) c -> p ho c", p=D))

            with (
                tc.tile_pool(name="xpool", bufs=2) as xpool,
                tc.tile_pool(name="tpool", bufs=2) as tpool,
                tc.tile_pool(name="rpool", bufs=3) as rpool,
                tc.tile_pool(name="qkout", bufs=3) as qkout,
                tc.tile_pool(name="ps1", bufs=4, space="PSUM") as ps1,
            ):
                for tci in range(T // 512):
                    t0 = tci * 512
                    x_tile = xpool.tile([D, H // D, 512], BF16, tag="x")
                    nc.sync.dma_start(
                        x_tile[:],
                        x_t[:, t0 : t0 + 512].rearrange("(ho p) t -> p ho t", p=D),
                    )
                    tb = tpool.tile([D, 4, 512], F32, tag="tb")
                    for ti in range(4):
                        nc.sync.dma_start(tb[:, ti, :], tabs[ti, :, t0 : t0 + 512])

                    for ct in range(2 * CH // D):  # 0-3: q heads, 4-7: k heads
                        psum = ps1.tile([D, 512], F32, tag="ps_qk")
                        for h in range(H // D):
                            nc.tensor.matmul(
                                psum[:],
                                wqk_sb[:, h, ct * D : (ct + 1) * D],
                                x_tile[:, h, :],
                                start=(h == 0),
                                stop=(h == H // D - 1),
                            )
                        is_q = ct < HEADS_PER_CORE
                        cos_i = 0 if is_q else 2
                        sin_i = 1 if is_q else 3
                        tmp1 = rpool.tile([D, 512], F32, tag="tmp1")
                        nc.vector.tensor_mul(tmp1[:], psum[:], tb[:, cos_i, :])
                        tmp2 = rpool.tile([D, 512], F32, tag="tmp2")
                        nc.vector.tensor_mul(
                            tmp2[0:64, :], psum[64:128, :], tb[0:64, sin_i, :]
                        )
                        nc.vector.tensor_mul(
                            tmp2[64:128, :], psum[0:64, :], tb[64:128, sin_i, :]
                        )
                        qk_bf = qkout.tile([D, 512], BF16, tag="qk_bf")
                        nc.vector.tensor_add(qk_bf[:], tmp1[:], tmp2[:])
                        head = ct % HEADS_PER_CORE
                        dst = qt_d if is_q else kt_d
                        nc.sync.dma_start(
                            dst[head * D : (head + 1) * D, t0 : t0 + 512], qk_bf[:]
                        )

            # ---------------- stage 1b: V projection ----------------
            wv_sb = wpool.tile([D, H // D, 2 * CH], BF16, tag="w", name="wv_sb")[:, :, :CH]
            nc.sync.dma_start(wv_sb[:], wv.rearrange("(ho p) c -> p ho c", p=D))
            with (
                tc.tile_pool(name="xpool2", bufs=2) as xpool2,
                tc.tile_pool(name="vout", bufs=3) as vout,
                tc.tile_pool(name="ps1b", bufs=4, space="PSUM") as ps1b,
            ):
                for tci in range(T // 512):
                    t0 = tci * 512
                    x_tile = xpool2.tile([D, H // D, 512], BF16, tag="x2")
                    nc.sync.dma_start(
                        x_tile[:],
                        x_t[:, t0 : t0 + 512].rearrange("(ho p) t -> p ho t", p=D),
                    )
                    for ts in range(4):
                        psum = ps1b.tile([D, 512], F32, tag="ps_v")
                        for h in range(H // D):
                            nc.tensor.matmul(
                                psum[:],
                                x_tile[:, h, ts * D : (ts + 1) * D],
                                wv_sb[:, h, :],
                                start=(h == 0),
                                stop=(h == H // D - 1),
                            )
                        v_bf = vout.tile([D, CH], BF16, tag="v_bf")
                        nc.vector.tensor_copy(v_bf[:], psum[:])
                        nc.sync.dma_start(
                            v_d[t0 + ts * D : t0 + (ts + 1) * D, :], v_bf[:]
                        )

            # ---------------- stage 2: attention per (batch, head) ----------------
            with (
                tc.tile_pool(name="kqv", bufs=2) as kqv,
                tc.tile_pool(name="ppool", bufs=4) as ppool,
                tc.tile_pool(name="mpool", bufs=3) as mpool,
                tc.tile_pool(name="epi", bufs=3) as epi,
                tc.tile_pool(name="ps_s", bufs=2, space="PSUM") as ps_s,
                tc.tile_pool(name="ps_av", bufs=2, space="PSUM") as ps_av,
                tc.tile_pool(name="ps_sum", bufs=2, space="PSUM") as ps_sum,
                tc.tile_pool(name="ps_bc", bufs=2, space="PSUM") as ps_bc,
            ):
                for b in range(B):
                    for head in range(HEADS_PER_CORE):
                        k_sb = kqv.tile([D, S], BF16, tag="k_sb")
                        nc.sync.dma_start(
                            k_sb[:],
                            kt_d[head * D : (head + 1) * D, b * S : (b + 1) * S],
                        )
                        q_sb = kqv.tile([D, S], BF16, tag="q_sb")
                        nc.sync.dma_start(
                            q_sb[:],
                            qt_d[head * D : (head + 1) * D, b * S : (b + 1) * S],
                        )
                        v_sb = kqv.tile([D, KT, D], BF16, tag="v_sb")
                        nc.sync.dma_start(
                            v_sb[:],
                            v_d[b * S : (b + 1) * S, head * D : (head + 1) * D].rearrange(
                                "(o p) c -> p o c", p=D
                            ),
                        )
                        for qc in range(QC):
                            kts = [kt for kt in range(KT) if ops[b, qc, kt] != SKIP]
                            psum_av = ps_av.tile([D, 512], F32, tag="av")
                            psum_sum = ps_sum.tile([1, 512], F32, tag="sm")
                            n_kt = len(kts)
                            for i, kt in enumerate(kts):
                                psum_s = ps_s.tile([D, 512], F32, tag="s")
                                nc.tensor.matmul(
                                    psum_s[:],
                                    k_sb[:, kt * D : (kt + 1) * D],
                                    q_sb[:, qc * 512 : (qc + 1) * 512],
                                    start=True,
                                    stop=True,
                                )
                                op = ops[b, qc, kt]
                                if op == DIAG:
                                    delta = kt * D - qc * 512
                                    nc.vector.tensor_add(
                                        psum_s[:],
                                        psum_s[:],
                                        maskfull[:, 384 - delta : 896 - delta],
                                    )
                                elif op == DMAMASK:
                                    mt = mpool.tile([D, 512], F32, tag="mt")
                                    nc.sync.dma_start(
                                        mt[:],
                                        maskT[
                                            b,
                                            kt * D : (kt + 1) * D,
                                            qc * 512 : (qc + 1) * 512,
                                        ],
                                    )
                                    nc.vector.tensor_add(psum_s[:], psum_s[:], mt[:])
                                pexp = ppool.tile([D, 512], BF16, tag="pexp")
                                nc.scalar.activation(
                                    pexp[:], psum_s[:], mybir.ActivationFunctionType.Exp
                                )
                                nc.tensor.matmul(
                                    psum_sum[:],
                                    ones_col[:],
                                    pexp[:],
                                    start=(i == 0),
                                    stop=(i == n_kt - 1),
                                )
                                nc.tensor.matmul(
                                    psum_av[:],
                                    v_sb[:, kt, :],
                                    pexp[:],
                                    start=(i == 0),
                                    stop=(i == n_kt - 1),
                                )
                            recip = epi.tile([1, 512], F32, tag="recip")
                            nc.vector.reciprocal(recip[:], psum_sum[:])
                            psum_bc = ps_bc.tile([D, 512], F32, tag="bc")
                            nc.tensor.matmul(
                                psum_bc[:], ones_row[:], recip[:], start=True, stop=True
                            )
                            bc_sb = epi.tile([D, 512], F32, tag="bc_sb")
                            nc.scalar.copy(bc_sb[:], psum_bc[:])
                            attn_sb = epi.tile([D, 512], BF16, tag="attn_sb")
                            nc.vector.tensor_mul(attn_sb[:], psum_av[:], bc_sb[:])
                            nc.sync.dma_start(
                                att_loc[b][
                                    head * D : (head + 1) * D, qc * 512 : (qc + 1) * 512
                                ],
                                attn_sb[:],
                            )
                    nc.gpsimd.collective_compute(
                        "AllGather",
                        mybir.AluOpType.bypass,
                        replica_groups=rg,
                        ins=[att_loc[b].opt()],
                        outs=[att_all[b].opt()],
                    )

            # ---------------- stage 3: o_proj slice ----------------
            wo_sb = wpool.tile([D, H // D, 2 * CH], BF16, tag="w", name="wo_sb")[:, :, :CH]
            nc.sync.dma_start(wo_sb[:], wo.rearrange("(co p) o -> p co o", p=D))
            with (
                tc.tile_pool(name="apool", bufs=3) as apool,
                tc.tile_pool(name="opool", bufs=3) as opool,
                tc.tile_pool(name="ps3", bufs=4, space="PSUM") as ps3,
            ):
                for b in range(B):
                    for tt in range(S // D):
                        a_sb = apool.tile([D, H // D, D], BF16, tag="a_sb")
                        nc.sync.dma_start(
                            a_sb[:],
                            att_all[b][:, tt * D : (tt + 1) * D].rearrange(
                                "(co p) t -> p co t", p=D
                            ),
                        )
                        psum_o = ps3.tile([D, CH], F32, tag="ps_o")
                        for ct in range(H // D):
                            nc.tensor.matmul(
                                psum_o[:],
                                a_sb[:, ct, :],
                                wo_sb[:, ct, :],
                                start=(ct == 0),
                                stop=(ct == H // D - 1),
                            )
                        o_sb = opool.tile([D, CH], F32, tag="o_sb")
                        nc.vector.tensor_copy(o_sb[:], psum_o[:])
                        nc.sync.dma_start(
                            out[b * S + tt * D : b * S + (tt + 1) * D, :], o_sb[:]
                        )

    nc.compile()
    return nc, maskT is not None


def kernel(hidden_states, attention_mask, position_ids, W_pack, W_o):
    _ensure_trace_hook()
    hidden_states = np.asarray(hidden_states, dtype=np.float32)
    attention_mask = np.asarray(attention_mask, dtype=np.float32)
    position_ids = np.asarray(position_ids)
    W_pack = np.asarray(W_pack, dtype=np.float32)
    W_o = np.asarray(W_o, dtype=np.float32)

    ops, need_dma = _classify_mask(attention_mask)

    key = (ops.tobytes(), need_dma)
    if key not in _cache:
        _cache.clear()
        _cache[key] = _build(ops, need_dma)
    nc, has_mask_param = _cache[key]

    # ---- host-side prep ----
    X_T = np.ascontiguousarray(hidden_states.reshape(T, H).T).astype(ml_dtypes.bfloat16)

    # RoPE tables (position-gathered), transposed to [d, t]; scale folded into Q's.
    pos = position_ids.reshape(T).astype(np.float32)
    inv_freq = (1.0 / (BASE ** (np.arange(0, D, 2, dtype=np.float32) / D))).astype(
        np.float32
    )
    ang = pos[:, None] * inv_freq[None, :]          # [T, 64]
    ang = np.concatenate([ang, ang], axis=1)         # [T, 128]
    cos = np.cos(ang).astype(np.float32)
    sin = np.sin(ang).astype(np.float32)
    sin_signed = sin.copy()
    sin_signed[:, :64] *= -1.0                       # rows d<64 multiply -q[d+64]
    isd = np.float32(1.0 / math.sqrt(D))
    tabs = np.stack(
        [
            (cos * isd).T,
            (sin_signed * isd).T,
            cos.T,
            sin_signed.T,
        ]
    ).astype(np.float32)                             # [4, 128, T]
    tabs = np.ascontiguousarray(tabs)

    maskT_np = None
    if has_mask_param:
        maskT_np = np.ascontiguousarray(
            np.transpose(attention_mask[:, 0], (0, 2, 1))
        ).astype(np.float32)                         # [B, S(k), S(q)]

    in_maps = []
    for c in range(N_CORES):
        qr = slice(c * CH, (c + 1) * CH)
        kr = slice(H + c * CH, H + (c + 1) * CH)
        vr = slice(2 * H + c * CH, 2 * H + (c + 1) * CH)
        wqk_c = np.ascontiguousarray(
            np.concatenate([W_pack[qr], W_pack[kr]], axis=0).T
        ).astype(ml_dtypes.bfloat16)                 # [H, 1024]
        wv_c = np.ascontiguousarray(W_pack[vr].T).astype(ml_dtypes.bfloat16)  # [H, 512]
        wo_c = np.ascontiguousarray(W_o[c * CH : (c + 1) * CH, :].T).astype(
            ml_dtypes.bfloat16
        )                                            # [H, 512]
        m = {"x_t": X_T, "wqk": wqk_c, "wv": wv_c, "wo": wo_c, "tabs": tabs}
        if has_mask_param:
            m["maskT"] = maskT_np
        in_maps.append(m)

    import os

    trace = bool(os.environ.get("BASS_TRACE"))
    res = run_bass_kernel_spmd(
        nc, in_maps, core_ids=list(range(N_CORES)), trace=trace
    )
    last_run_info["exec_time_ns"] = res.exec_time_ns
    last_run_info["profile_json"] = getattr(res, "profile_json", None)

    outs = [res.results[c]["out"].reshape(B, S, CH) for c in range(N_CORES)]
    return np.concatenate(outs, axis=2)


# revision 5
# speedup vs baseline: 1.0261x; 1.0261x over previous
"""Distributed Bass kernel for nn_Attention (dense transformer attention block).

Tensor-parallel over heads across 8 TRN2 NeuronCores:
  - each core owns 4 heads: its slice of W_pack (QKV) and the matching
    512 input channels of o_proj,
  - attention (RoPE + causal softmax) is computed fully locally per head,
  - attention outputs are AllGathered (bf16) and each core computes a
    distinct 512-column slice of the o_proj output; the host concatenates
    the slices (no AllReduce needed).

Compute dtype: bf16 matmuls with f32 PSUM accumulation, f32 softmax.
Scores are computed transposed ([k, q] layout) so the softmax exp writes
P^T directly and PV needs no transpose; row sums come from a ones-vector
matmul and the normalization is applied to the (much smaller) attention
output via a K=1 broadcast matmul.
"""

import sys
import types
import math

sys.path.insert(0, "/opt/trn_rl_repo")

import numpy as np
import ml_dtypes

from concourse import bacc, tile, mybir
from concourse.bass_utils import run_bass_kernel_spmd

BF16 = mybir.dt.bfloat16
F32 = mybir.dt.float32

B = 2
S = 2048
H = 4096
NH = 32
D = 128
T = B * S
N_CORES = 8
HEADS_PER_CORE = NH // N_CORES          # 4
CH = HEADS_PER_CORE * D                 # 512 channels per core
BASE = 10000.0
NEG = np.float32(np.finfo(np.float32).min)

# mask-block ops (per [q-chunk=512, k-tile=128] block, scoresT layout)
SKIP, NOMASK, DIAG, DMAMASK = 0, 1, 2, 3

QC = S // 512                            # 4 q-chunks per batch
KT = S // 128                            # 16 k-tiles per batch

_cache = {}
last_run_info = {}


def _ensure_trace_hook():
    """Register the NTFF profile hook missing from this image's antenv."""
    if "antenv.axon_hooks" in sys.modules:
        return
    try:
        from trn_agent_boot.trn_boot import _ntff_profile_via_ctypes

        hook = _ntff_profile_via_ctypes("/opt/axon/libaxon_pjrt.so")
        mod = types.ModuleType("antenv.axon_hooks")
        mod.get_axon_ntff_profile_hook = lambda: hook
        mod.set_axon_ntff_profile_hook = lambda h: None
        sys.modules["antenv.axon_hooks"] = mod
        from concourse import bass_utils

        bass_utils.upload_artifacts = lambda tmpdir: tmpdir
    except Exception:
        pass


def _classify_mask(mask):
    """Per (b, q-chunk 512, k-tile 128) block op for the scoresT layout."""
    ops = np.empty((B, QC, KT), dtype=np.int32)
    karr = np.arange(128)
    qarr = np.arange(512)
    need_dma = False
    for b in range(B):
        mb = np.asarray(mask[b, 0])
        for qc in range(QC):
            qs = qc * 512
            for kt in range(KT):
                ks = kt * 128
                sub = mb[qs : qs + 512, ks : ks + 128]  # [q, k]
                if np.all(sub <= -1e30):
                    ops[b, qc, kt] = SKIP
                elif not sub.any():
                    ops[b, qc, kt] = NOMASK
                else:
                    delta = ks - qs
                    if 0 <= delta <= 384:
                        pat = np.where(
                            (delta + karr[None, :]) > qarr[:, None], NEG, np.float32(0)
                        )
                        if np.array_equal(sub, pat):
                            ops[b, qc, kt] = DIAG
                            continue
                    ops[b, qc, kt] = DMAMASK
                    need_dma = True
    return ops, need_dma


def _build(ops, need_dma):
    nc = bacc.Bacc(None, target_bir_lowering=False)

    x_t = nc.declare_dram_parameter("x_t", [H, T], BF16, isOutput=False)
    wqk = nc.declare_dram_parameter("wqk", [H, 2 * CH], BF16, isOutput=False)
    wv = nc.declare_dram_parameter("wv", [H, CH], BF16, isOutput=False)
    wo = nc.declare_dram_parameter("wo", [H, CH], BF16, isOutput=False)
    tabs = nc.declare_dram_parameter("tabs", [4, D, T], F32, isOutput=False)
    maskT = None
    if need_dma:
        maskT = nc.declare_dram_parameter("maskT", [B, S, S], F32, isOutput=False)
    out = nc.declare_dram_parameter("out", [T, CH], F32, isOutput=True)

    ones_col_np = np.ones((D, 1), dtype=ml_dtypes.bfloat16)
    ones_row_np = np.ones((1, D), dtype=np.float32)
    # maskfull[k, j] = NEG if k > j - 384 else 0   (slice [384-delta : 896-delta])
    j = np.arange(896)
    maskfull_np = np.where(np.arange(D)[:, None] > (j[None, :] - 384), NEG, 0.0).astype(
        np.float32
    )

    rg = [list(range(N_CORES))]
    NHT = H // D  # 32 h-tiles

    with tile.TileContext(nc) as tc:
        with (
            tc.tile_pool(name="dram", bufs=1, space="DRAM") as dram,
            tc.tile_pool(name="const", bufs=1) as constp,
            tc.tile_pool(name="wpool", bufs=1) as wpool,
        ):
            qt_d = dram.tile([CH, T], BF16, tag="qt_d")
            kt_d = dram.tile([CH, T], BF16, tag="kt_d")
            v_d = dram.tile([T, CH], BF16, tag="v_d")
            att_loc = [
                dram.tile([CH, S], BF16, tag=f"att_loc{b}", name=f"att_loc{b}")
                for b in range(B)
            ]
            att_all = [
                dram.tile(
                    [N_CORES * CH, S],
                    BF16,
                    addr_space="Shared",
                    tag=f"att_all{b}",
                    name=f"att_all{b}",
                )
                for b in range(B)
            ]

            ones_col = constp.tile([D, 1], BF16, tag="ones_col")
            nc.sync.dma_start(ones_col[:], nc.inline_tensor(ones_col_np, "ones_col_c")[:])
            ones_row = constp.tile([1, D], F32, tag="ones_row")
            nc.sync.dma_start(ones_row[:], nc.inline_tensor(ones_row_np, "ones_row_c")[:])
            maskfull = constp.tile([D, 896], F32, tag="maskfull")
            nc.sync.dma_start(maskfull[:], nc.inline_tensor(maskfull_np, "maskfull_c")[:])

            # ------------- stage 1: QKV projection + RoPE (single X pass) -------------
            wqk_sb = wpool.tile([D, NHT, 2 * CH], BF16, tag="wqk")
            nc.sync.dma_start(wqk_sb[:], wqk.rearrange("(ho p) c -> p ho c", p=D))
            wv_sb = wpool.tile([D, NHT, CH], BF16, tag="wv")
            nc.sync.dma_start(wv_sb[:], wv.rearrange("(ho p) c -> p ho c", p=D))

            with (
                tc.tile_pool(name="xpool", bufs=2) as xpool,
                tc.tile_pool(name="tpool", bufs=1) as tpool,
                tc.tile_pool(name="rpool", bufs=2) as rpool,
                tc.tile_pool(name="qkout", bufs=2) as qkout,
                tc.tile_pool(name="ps1", bufs=6, space="PSUM") as ps1,
            ):
                for tci in range(T // 512):
                    t0 = tci * 512
                    x_tile = xpool.tile([D, NHT, 512], BF16, tag="x")
                    nc.sync.dma_start(
                        x_tile[:],
                        x_t[:, t0 : t0 + 512].rearrange("(ho p) t -> p ho t", p=D),
                    )
                    tb = tpool.tile([D, 4, 512], F32, tag="tb")
                    for ti in range(4):
                        nc.sync.dma_start(tb[:, ti, :], tabs[ti, :, t0 : t0 + 512])

                    for ct in range(2 * CH // D):  # 0-3: q heads, 4-7: k heads
                        psum = ps1.tile([D, 512], F32, tag="ps1")
                        for h in range(NHT):
                            nc.tensor.matmul(
                                psum[:],
                                wqk_sb[:, h, ct * D : (ct + 1) * D],
                                x_tile[:, h, :],
                                start=(h == 0),
                                stop=(h == NHT - 1),
                            )
                        is_q = ct < HEADS_PER_CORE
                        cos_i = 0 if is_q else 2
                        sin_i = 1 if is_q else 3
                        tmp1 = rpool.tile([D, 512], F32, tag="tmp1")
                        nc.vector.tensor_mul(tmp1[:], psum[:], tb[:, cos_i, :])
                        tmp2 = rpool.tile([D, 512], F32, tag="tmp2")
                        nc.vector.tensor_mul(
                            tmp2[0:64, :], psum[64:128, :], tb[0:64, sin_i, :]
                        )
                        nc.vector.tensor_mul(
                            tmp2[64:128, :], psum[0:64, :], tb[64:128, sin_i, :]
                        )
                        qk_bf = qkout.tile([D, 512], BF16, tag="qk_bf")
                        nc.vector.tensor_add(qk_bf[:], tmp1[:], tmp2[:])
                        head = ct % HEADS_PER_CORE
                        dst = qt_d if is_q else kt_d
                        nc.sync.dma_start(
                            dst[head * D : (head + 1) * D, t0 : t0 + 512], qk_bf[:]
                        )

                    for ts in range(4):  # V: [t, ch] layout
                        psum = ps1.tile([D, 512], F32, tag="ps1", name="psum_v")
                        for h in range(NHT):
                            nc.tensor.matmul(
                                psum[:],
                                x_tile[:, h, ts * D : (ts + 1) * D],
                                wv_sb[:, h, :],
                                start=(h == 0),
                                stop=(h == NHT - 1),
                            )
                        v_bf = qkout.tile([D, CH], BF16, tag="v_bf")
                        nc.vector.tensor_copy(v_bf[:], psum[:])
                        nc.sync.dma_start(
                            v_d[t0 + ts * D : t0 + (ts + 1) * D, :], v_bf[:]
                        )

            # ------------- stage 2: attention per (batch, head) -------------
            with (
                tc.tile_pool(name="kqv", bufs=2) as kqv,
                tc.tile_pool(name="ppool", bufs=6) as ppool,
                tc.tile_pool(name="mpool", bufs=3) as mpool,
                tc.tile_pool(name="epi", bufs=3) as epi,
                tc.tile_pool(name="ps_s", bufs=3, space="PSUM") as ps_s,
                tc.tile_pool(name="ps_av", bufs=2, space="PSUM") as ps_av,
                tc.tile_pool(name="ps_sum", bufs=1, space="PSUM") as ps_sum,
                tc.tile_pool(name="ps_bc", bufs=2, space="PSUM") as ps_bc,
            ):
                pending = []  # deferred per-kt tails and epilogues

                def drain_pending(n=None):
                    todo = pending[:] if n is None else pending[:n]
                    del pending[: len(todo)]
                    for fn in todo:
                        fn()

                for b in range(B):
                    for head in range(HEADS_PER_CORE):
                        k_sb = kqv.tile([D, S], BF16, tag="k_sb")
                        nc.sync.dma_start(
                            k_sb[:],
                            kt_d[head * D : (head + 1) * D, b * S : (b + 1) * S],
                        )
                        q_sb = kqv.tile([D, S], BF16, tag="q_sb")
                        nc.sync.dma_start(
                            q_sb[:],
                            qt_d[head * D : (head + 1) * D, b * S : (b + 1) * S],
                        )
                        v_sb = kqv.tile([D, KT, D], BF16, tag="v_sb")
                        nc.sync.dma_start(
                            v_sb[:],
                            v_d[
                                b * S : (b + 1) * S, head * D : (head + 1) * D
                            ].rearrange("(o p) c -> p o c", p=D),
                        )
                        for qc in range(QC):
                            kts = [kt for kt in range(KT) if ops[b, qc, kt] != SKIP]
                            psum_av = ps_av.tile([D, 512], F32, tag="av")
                            psum_sum = ps_sum.tile([1, 512], F32, tag="sm")
                            n_kt = len(kts)
                            for i, kt in enumerate(kts):
                                psum_s = ps_s.tile([D, 512], F32, tag="s")
                                nc.tensor.matmul(
                                    psum_s[:],
                                    k_sb[:, kt * D : (kt + 1) * D],
                                    q_sb[:, qc * 512 : (qc + 1) * 512],
                                    start=True,
                                    stop=True,
                                )
                                op = ops[b, qc, kt]
                                if op == DIAG:
                                    delta = kt * D - qc * 512
                                    nc.vector.tensor_add(
                                        psum_s[:],
                                        psum_s[:],
                                        maskfull[:, 384 - delta : 896 - delta],
                                    )
                                elif op == DMAMASK:
                                    mt = mpool.tile([D, 512], F32, tag="mt")
                                    nc.sync.dma_start(
                                        mt[:],
                                        maskT[
                                            b,
                                            kt * D : (kt + 1) * D,
                                            qc * 512 : (qc + 1) * 512,
                                        ],
                                    )
                                    nc.vector.tensor_add(psum_s[:], psum_s[:], mt[:])

                                def tail(
                                    psum_s=psum_s,
                                    psum_av=psum_av,
                                    psum_sum=psum_sum,
                                    i=i,
                                    kt=kt,
                                    n_kt=n_kt,
                                    v_sb=v_sb,
                                ):
                                    pexp = ppool.tile(
                                        [D, 512], BF16, tag="pexp", name="pexp"
                                    )
                                    nc.scalar.activation(
                                        pexp[:],
                                        psum_s[:],
                                        mybir.ActivationFunctionType.Exp,
                                    )
                                    nc.tensor.matmul(
                                        psum_sum[:],
                                        ones_col[:],
                                        pexp[:],
                                        start=(i == 0),
                                        stop=(i == n_kt - 1),
                                    )
                                    nc.tensor.matmul(
                                        psum_av[:],
                                        v_sb[:, kt, :],
                                        pexp[:],
                                        start=(i == 0),
                                        stop=(i == n_kt - 1),
                                    )

                                pending.append(tail)
                                # keep ~2 score-matmuls in flight ahead of the tails
                                if len(pending) > 2:
                                    drain_pending(len(pending) - 2)

                            def epilogue(
                                psum_av=psum_av,
                                psum_sum=psum_sum,
                                b=b,
                                head=head,
                                qc=qc,
                            ):
                                recip = epi.tile([1, 512], F32, tag="recip", name="recip")
                                nc.vector.reciprocal(recip[:], psum_sum[:])
                                psum_bc = ps_bc.tile([D, 512], F32, tag="bc", name="bc")
                                nc.tensor.matmul(
                                    psum_bc[:], ones_row[:], recip[:], start=True, stop=True
                                )
                                bc_sb = epi.tile([D, 512], F32, tag="bc_sb", name="bc_sb")
                                nc.scalar.copy(bc_sb[:], psum_bc[:])
                                attn_sb = epi.tile(
                                    [D, 512], BF16, tag="attn_sb", name="attn_sb"
                                )
                                nc.vector.tensor_mul(attn_sb[:], psum_av[:], bc_sb[:])
                                nc.sync.dma_start(
                                    att_loc[b][
                                        head * D : (head + 1) * D,
                                        qc * 512 : (qc + 1) * 512,
                                    ],
                                    attn_sb[:],
                                )

                            pending.append(epilogue)

                    # all heads of batch b issued; flush and gather
                    drain_pending()
                    nc.gpsimd.collective_compute(
                        "AllGather",
                        mybir.AluOpType.bypass,
                        replica_groups=rg,
                        ins=[att_loc[b].opt()],
                        outs=[att_all[b].opt()],
                    )

            # ------------- stage 3: o_proj slice -------------
            wo_sb = wpool.tile([D, NHT, 2 * CH], BF16, tag="wqk", name="wo_sb")[:, :, :CH]
            nc.sync.dma_start(wo_sb[:], wo.rearrange("(co p) o -> p co o", p=D))
            with (
                tc.tile_pool(name="apool", bufs=3) as apool,
                tc.tile_pool(name="opool", bufs=3) as opool,
                tc.tile_pool(name="ps3", bufs=4, space="PSUM") as ps3,
            ):
                for b in range(B):
                    for tt in range(S // D):
                        a_sb = apool.tile([D, NHT, D], BF16, tag="a_sb")
                        nc.sync.dma_start(
                            a_sb[:],
                            att_all[b][:, tt * D : (tt + 1) * D].rearrange(
                                "(co p) t -> p co t", p=D
                            ),
                        )
                        psum_o = ps3.tile([D, CH], F32, tag="ps_o")
                        for ct in range(NHT):
                            nc.tensor.matmul(
                                psum_o[:],
                                a_sb[:, ct, :],
                                wo_sb[:, ct, :],
                                start=(ct == 0),
                                stop=(ct == NHT - 1),
                            )
                        o_sb = opool.tile([D, CH], F32, tag="o_sb")
                        nc.vector.tensor_copy(o_sb[:], psum_o[:])
                        nc.sync.dma_start(
                            out[b * S + tt * D : b * S + (tt + 1) * D, :], o_sb[:]
                        )

    nc.compile()
    return nc, maskT is not None


def kernel(hidden_states, attention_mask, position_ids, W_pack, W_o):
    _ensure_trace_hook()
    hidden_states = np.asarray(hidden_states, dtype=np.float32)
    attention_mask = np.asarray(attention_mask, dtype=np.float32)
    position_ids = np.asarray(position_ids)
    W_pack = np.asarray(W_pack, dtype=np.float32)
    W_o = np.asarray(W_o, dtype=np.float32)

    ops, need_dma = _classify_mask(attention_mask)

    key = (ops.tobytes(), need_dma)
    if key not in _cache:
        _cache.clear()
        _cache[key] = _build(ops, need_dma)
    nc, has_mask_param = _cache[key]

    # ---- host-side prep ----
    X_T = np.ascontiguousarray(hidden_states.reshape(T, H).T).astype(ml_dtypes.bfloat16)

    # RoPE tables (position-gathered), transposed to [d, t]; scale folded into Q's.
    pos = position_ids.reshape(T).astype(np.float32)
    inv_freq = (1.0 / (BASE ** (np.arange(0, D, 2, dtype=np.float32) / D))).astype(
        np.float32
    )
    ang = pos[:, None] * inv_freq[None, :]          # [T, 64]
    ang = np.concatenate([ang, ang], axis=1)         # [T, 128]
    cos = np.cos(ang).astype(np.float32)
    sin = np.sin(ang).astype(np.float32)
    sin_signed = sin.copy()
    sin_signed[:, :64] *= -1.0                       # rows d<64 multiply -q[d+64]
    isd = np.float32(1.0 / math.sqrt(D))
    tabs = np.stack(
        [
            (cos * isd).T,
            (sin_signed * isd).T,
            cos.T,
            sin_signed.T,
        ]
    ).astype(np.float32)                             # [4, 128, T]
    tabs = np.ascontiguousarray(tabs)

    maskT_np = None
    if has_mask_param:
        maskT_np = np.ascontiguousarray(
            np.transpose(attention_mask[:, 0], (0, 2, 1))
        ).astype(np.float32)                         # [B, S(k), S(q)]

    in_maps = []
    for c in range(N_CORES):
        qr = slice(c * CH, (c + 1) * CH)
        kr = slice(H + c * CH, H + (c + 1) * CH)
        vr = slice(2 * H + c * CH, 2 * H + (c + 1) * CH)
        wqk_c = np.ascontiguousarray(
            np.concatenate([W_pack[qr], W_pack[kr]], axis=0).T
        ).astype(ml_dtypes.bfloat16)                 # [H, 1024]
        wv_c = np.ascontiguousarray(W_pack[vr].T).astype(ml_dtypes.bfloat16)  # [H, 512]
        wo_c = np.ascontiguousarray(W_o[c * CH : (c + 1) * CH, :].T).astype(
            ml_dtypes.bfloat16
        )                                            # [H, 512]
        m = {"x_t": X_T, "wqk": wqk_c, "wv": wv_c, "wo": wo_c, "tabs": tabs}
        if has_mask_param:
            m["maskT"] = maskT_np
        in_maps.append(m)

    import os

    trace = bool(os.environ.get("BASS_TRACE"))
    res = run_bass_kernel_spmd(
        nc, in_maps, core_ids=list(range(N_CORES)), trace=trace
    )
    last_run_info["exec_time_ns"] = res.exec_time_ns
    last_run_info["profile_json"] = getattr(res, "profile_json", None)

    outs = [res.results[c]["out"].reshape(B, S, CH) for c in range(N_CORES)]
    return np.concatenate(outs, axis=2)


# revision 8
# speedup vs baseline: 1.1231x; 1.0946x over previous
"""Distributed Bass kernel for nn_Attention (dense transformer attention block).

Tensor-parallel over heads across 8 TRN2 NeuronCores:
  - each core owns 4 heads: its slice of W_pack (QKV) and the matching
    512 input channels of o_proj,
  - attention (RoPE + causal softmax) is computed fully locally per head,
  - attention outputs are AllGathered (bf16) and each core computes a
    distinct 512-column slice of the o_proj output; the host concatenates
    the slices (no AllReduce needed).

Compute dtype: bf16 matmuls with f32 PSUM accumulation, f32 softmax.
Scores are computed transposed ([k, q] layout) so the softmax exp writes
P^T directly and PV needs no transpose; row sums come from a ones-vector
matmul and the normalization is applied to the (much smaller) attention
output via a K=1 broadcast matmul.
"""

import sys
import types
import math

sys.path.insert(0, "/opt/trn_rl_repo")

import numpy as np
import ml_dtypes

from concourse import bacc, tile, mybir
from concourse.bass_utils import run_bass_kernel_spmd

BF16 = mybir.dt.bfloat16
F32 = mybir.dt.float32

B = 2
S = 2048
H = 4096
NH = 32
D = 128
T = B * S
N_CORES = 8
HEADS_PER_CORE = NH // N_CORES          # 4
CH = HEADS_PER_CORE * D                 # 512 channels per core
BASE = 10000.0
NEG = np.float32(np.finfo(np.float32).min)

# mask-block ops (per [q-chunk=512, k-tile=128] block, scoresT layout)
SKIP, NOMASK, DIAG, DMAMASK = 0, 1, 2, 3

QC = S // 512                            # 4 q-chunks per batch
KT = S // 128                            # 16 k-tiles per batch

_cache = {}
last_run_info = {}


def _ensure_trace_hook():
    """Register the NTFF profile hook missing from this image's antenv."""
    if "antenv.axon_hooks" in sys.modules:
        return
    try:
        from trn_agent_boot.trn_boot import _ntff_profile_via_ctypes

        hook = _ntff_profile_via_ctypes("/opt/axon/libaxon_pjrt.so")
        mod = types.ModuleType("antenv.axon_hooks")
        mod.get_axon_ntff_profile_hook = lambda: hook
        mod.set_axon_ntff_profile_hook = lambda h: None
        sys.modules["antenv.axon_hooks"] = mod
        from concourse import bass_utils

        bass_utils.upload_artifacts = lambda tmpdir: tmpdir
    except Exception:
        pass


def _classify_mask(mask):
    """Per (b, q-chunk 512, k-tile 128) block op for the scoresT layout."""
    ops = np.empty((B, QC, KT), dtype=np.int32)
    karr = np.arange(128)
    qarr = np.arange(512)
    need_dma = False
    for b in range(B):
        mb = np.asarray(mask[b, 0])
        for qc in range(QC):
            qs = qc * 512
            for kt in range(KT):
                ks = kt * 128
                sub = mb[qs : qs + 512, ks : ks + 128]  # [q, k]
                if np.all(sub <= -1e30):
                    ops[b, qc, kt] = SKIP
                elif not sub.any():
                    ops[b, qc, kt] = NOMASK
                else:
                    delta = ks - qs
                    if 0 <= delta <= 384:
                        pat = np.where(
                            (delta + karr[None, :]) > qarr[:, None], NEG, np.float32(0)
                        )
                        if np.array_equal(sub, pat):
                            ops[b, qc, kt] = DIAG
                            continue
                    ops[b, qc, kt] = DMAMASK
                    need_dma = True
    return ops, need_dma


def _build(ops, need_dma):
    nc = bacc.Bacc(None, target_bir_lowering=False)

    x_t = nc.declare_dram_parameter("x_t", [H, T], BF16, isOutput=False)
    wqk = nc.declare_dram_parameter("wqk", [H, 2 * CH], BF16, isOutput=False)
    wv = nc.declare_dram_parameter("wv", [H, CH], BF16, isOutput=False)
    wo = nc.declare_dram_parameter("wo", [H, CH], BF16, isOutput=False)
    tabs = nc.declare_dram_parameter("tabs", [4, D, T], F32, isOutput=False)
    maskT = None
    if need_dma:
        maskT = nc.declare_dram_parameter("maskT", [B, S, S], F32, isOutput=False)
    out = nc.declare_dram_parameter("out", [T, CH], F32, isOutput=True)

    ones_col_np = np.ones((D, 1), dtype=ml_dtypes.bfloat16)
    ones_row_np = np.ones((1, D), dtype=np.float32)
    # maskfull[k, j] = NEG if k > j - 384 else 0   (slice [384-delta : 896-delta])
    j = np.arange(896)
    maskfull_np = np.where(np.arange(D)[:, None] > (j[None, :] - 384), NEG, 0.0).astype(
        np.float32
    )

    rg = [list(range(N_CORES))]
    NHT = H // D  # 32 h-tiles

    with tile.TileContext(nc) as tc:
        with (
            tc.tile_pool(name="dram", bufs=1, space="DRAM") as dram,
            tc.tile_pool(name="const", bufs=1) as constp,
            tc.tile_pool(name="wpool", bufs=1) as wpool,
        ):
            qt_d = dram.tile([CH, T], BF16, tag="qt_d")
            kt_d = dram.tile([CH, T], BF16, tag="kt_d")
            v_d = dram.tile([T, CH], BF16, tag="v_d")
            att_loc = [
                dram.tile([CH, S], BF16, tag=f"att_loc{b}", name=f"att_loc{b}")
                for b in range(B)
            ]
            att_all = [
                dram.tile(
                    [N_CORES * CH, S],
                    BF16,
                    addr_space="Shared",
                    tag=f"att_all{b}",
                    name=f"att_all{b}",
                )
                for b in range(B)
            ]

            ones_col = constp.tile([D, 1], BF16, tag="ones_col")
            nc.sync.dma_start(ones_col[:], nc.inline_tensor(ones_col_np, "ones_col_c")[:])
            ones_row = constp.tile([1, D], F32, tag="ones_row")
            nc.sync.dma_start(ones_row[:], nc.inline_tensor(ones_row_np, "ones_row_c")[:])
            maskfull = constp.tile([D, 896], F32, tag="maskfull")
            nc.sync.dma_start(maskfull[:], nc.inline_tensor(maskfull_np, "maskfull_c")[:])

            # ------------- stage 1: QKV projection + RoPE (single X pass) -------------
            wqk_sb = wpool.tile([D, NHT, 2 * CH], BF16, tag="wqk")
            nc.sync.dma_start(wqk_sb[:], wqk.rearrange("(ho p) c -> p ho c", p=D))
            wv_sb = wpool.tile([D, NHT, CH], BF16, tag="wv")
            nc.sync.dma_start(wv_sb[:], wv.rearrange("(ho p) c -> p ho c", p=D))

            with (
                tc.tile_pool(name="xpool", bufs=2) as xpool,
                tc.tile_pool(name="tpool", bufs=1) as tpool,
                tc.tile_pool(name="rpool", bufs=2) as rpool,
                tc.tile_pool(name="qkout", bufs=2) as qkout,
                tc.tile_pool(name="ps1", bufs=6, space="PSUM") as ps1,
            ):
                for tci in range(T // 512):
                    t0 = tci * 512
                    x_tile = xpool.tile([D, NHT, 512], BF16, tag="x")
                    nc.sync.dma_start(
                        x_tile[:],
                        x_t[:, t0 : t0 + 512].rearrange("(ho p) t -> p ho t", p=D),
                    )
                    tb = tpool.tile([D, 4, 512], F32, tag="tb")
                    for ti in range(4):
                        nc.sync.dma_start(tb[:, ti, :], tabs[ti, :, t0 : t0 + 512])

                    for ct in range(2 * CH // D):  # 0-3: q heads, 4-7: k heads
                        psum = ps1.tile([D, 512], F32, tag="ps1")
                        for h in range(NHT):
                            nc.tensor.matmul(
                                psum[:],
                                wqk_sb[:, h, ct * D : (ct + 1) * D],
                                x_tile[:, h, :],
                                start=(h == 0),
                                stop=(h == NHT - 1),
                            )
                        is_q = ct < HEADS_PER_CORE
                        cos_i = 0 if is_q else 2
                        sin_i = 1 if is_q else 3
                        tmp1 = rpool.tile([D, 512], F32, tag="tmp1")
                        nc.vector.tensor_mul(tmp1[:], psum[:], tb[:, cos_i, :])
                        tmp2 = rpool.tile([D, 512], F32, tag="tmp2")
                        nc.vector.tensor_mul(
                            tmp2[0:64, :], psum[64:128, :], tb[0:64, sin_i, :]
                        )
                        nc.vector.tensor_mul(
                            tmp2[64:128, :], psum[0:64, :], tb[64:128, sin_i, :]
                        )
                        qk_bf = qkout.tile([D, 512], BF16, tag="qk_bf")
                        nc.vector.tensor_add(qk_bf[:], tmp1[:], tmp2[:])
                        head = ct % HEADS_PER_CORE
                        dst = qt_d if is_q else kt_d
                        nc.sync.dma_start(
                            dst[head * D : (head + 1) * D, t0 : t0 + 512], qk_bf[:]
                        )

                    for ts in range(4):  # V: [t, ch] layout
                        psum = ps1.tile([D, 512], F32, tag="ps1", name="psum_v")
                        for h in range(NHT):
                            nc.tensor.matmul(
                                psum[:],
                                x_tile[:, h, ts * D : (ts + 1) * D],
                                wv_sb[:, h, :],
                                start=(h == 0),
                                stop=(h == NHT - 1),
                            )
                        v_bf = qkout.tile([D, CH], BF16, tag="v_bf")
                        nc.vector.tensor_copy(v_bf[:], psum[:])
                        nc.sync.dma_start(
                            v_d[t0 + ts * D : t0 + (ts + 1) * D, :], v_bf[:]
                        )

            # ------------- stage 2: attention per (batch, head) -------------
            with (
                tc.tile_pool(name="kqv", bufs=2) as kqv,
                tc.tile_pool(name="ppool", bufs=6) as ppool,
                tc.tile_pool(name="mpool", bufs=3) as mpool,
                tc.tile_pool(name="epi", bufs=3) as epi,
                tc.tile_pool(name="ps_s", bufs=3, space="PSUM") as ps_s,
                tc.tile_pool(name="ps_av", bufs=2, space="PSUM") as ps_av,
                tc.tile_pool(name="ps_sum", bufs=2, space="PSUM") as ps_sum,
                tc.tile_pool(name="ps_bc", bufs=1, space="PSUM") as ps_bc,
            ):
                pending = []  # deferred per-kt tails and epilogues

                def drain_pending(n=None):
                    todo = pending[:] if n is None else pending[:n]
                    del pending[: len(todo)]
                    for fn in todo:
                        fn()

                for b in range(B):
                    for head in range(HEADS_PER_CORE):
                        k_sb = kqv.tile([D, S], BF16, tag="k_sb")
                        nc.sync.dma_start(
                            k_sb[:],
                            kt_d[head * D : (head + 1) * D, b * S : (b + 1) * S],
                        )
                        q_sb = kqv.tile([D, S], BF16, tag="q_sb")
                        nc.sync.dma_start(
                            q_sb[:],
                            qt_d[head * D : (head + 1) * D, b * S : (b + 1) * S],
                        )
                        v_sb = kqv.tile([D, KT, D], BF16, tag="v_sb")
                        nc.sync.dma_start(
                            v_sb[:],
                            v_d[
                                b * S : (b + 1) * S, head * D : (head + 1) * D
                            ].rearrange("(o p) c -> p o c", p=D),
                        )
                        for qc in range(QC):
                            kts = [kt for kt in range(KT) if ops[b, qc, kt] != SKIP]
                            psum_av = ps_av.tile([D, 512], F32, tag="av")
                            psum_sum = ps_sum.tile([1, 512], F32, tag="sm")
                            n_kt = len(kts)
                            for i, kt in enumerate(kts):
                                psum_s = ps_s.tile([D, 512], F32, tag="s")
                                nc.tensor.matmul(
                                    psum_s[:],
                                    k_sb[:, kt * D : (kt + 1) * D],
                                    q_sb[:, qc * 512 : (qc + 1) * 512],
                                    start=True,
                                    stop=True,
                                )
                                op = ops[b, qc, kt]
                                if op == DIAG:
                                    delta = kt * D - qc * 512
                                    nc.vector.tensor_add(
                                        psum_s[:],
                                        psum_s[:],
                                        maskfull[:, 384 - delta : 896 - delta],
                                    )
                                elif op == DMAMASK:
                                    mt = mpool.tile([D, 512], F32, tag="mt")
                                    nc.sync.dma_start(
                                        mt[:],
                                        maskT[
                                            b,
                                            kt * D : (kt + 1) * D,
                                            qc * 512 : (qc + 1) * 512,
                                        ],
                                    )
                                    nc.vector.tensor_add(psum_s[:], psum_s[:], mt[:])

                                def tail(
                                    psum_s=psum_s,
                                    psum_av=psum_av,
                                    psum_sum=psum_sum,
                                    i=i,
                                    kt=kt,
                                    n_kt=n_kt,
                                    v_sb=v_sb,
                                ):
                                    pexp = ppool.tile(
                                        [D, 512], BF16, tag="pexp", name="pexp"
                                    )
                                    nc.scalar.activation(
                                        pexp[:],
                                        psum_s[:],
                                        mybir.ActivationFunctionType.Exp,
                                    )
                                    nc.tensor.matmul(
                                        psum_sum[:],
                                        ones_col[:],
                                        pexp[:],
                                        start=(i == 0),
                                        stop=(i == n_kt - 1),
                                    )
                                    nc.tensor.matmul(
                                        psum_av[:],
                                        v_sb[:, kt, :],
                                        pexp[:],
                                        start=(i == 0),
                                        stop=(i == n_kt - 1),
                                    )

                                pending.append(tail)
                                # keep ~2 score-matmuls in flight ahead of the tails
                                if len(pending) > 2:
                                    drain_pending(len(pending) - 2)

                            def epilogue(
                                psum_av=psum_av,
                                psum_sum=psum_sum,
                                b=b,
                                head=head,
                                qc=qc,
                            ):
                                sums_sb = epi.tile([1, 512], F32, tag="sums_sb", name="sums_sb")
                                nc.scalar.copy(sums_sb[:], psum_sum[:])
                                psum_bc = ps_bc.tile([D, 512], F32, tag="bc", name="bc")
                                nc.tensor.matmul(
                                    psum_bc[:], ones_row[:], sums_sb[:], start=True, stop=True
                                )
                                bc_sb = epi.tile([D, 512], F32, tag="bc_sb", name="bc_sb")
                                nc.vector.reciprocal_approx_fast(bc_sb[:], psum_bc[:])
                                attn_sb = epi.tile(
                                    [D, 512], BF16, tag="attn_sb", name="attn_sb"
                                )
                                nc.vector.tensor_mul(attn_sb[:], psum_av[:], bc_sb[:])
                                nc.sync.dma_start(
                                    att_loc[b][
                                        head * D : (head + 1) * D,
                                        qc * 512 : (qc + 1) * 512,
                                    ],
                                    attn_sb[:],
                                )

                            pending.append(epilogue)

                    # all heads of batch b issued; flush and gather
                    drain_pending()
                    nc.gpsimd.collective_compute(
                        "AllGather",
                        mybir.AluOpType.bypass,
                        replica_groups=rg,
                        ins=[att_loc[b].opt()],
                        outs=[att_all[b].opt()],
                    )

            # ------------- stage 3: o_proj slice -------------
            wo_sb = wpool.tile([D, NHT, 2 * CH], BF16, tag="wqk", name="wo_sb")[:, :, :CH]
            nc.sync.dma_start(wo_sb[:], wo.rearrange("(co p) o -> p co o", p=D))
            with (
                tc.tile_pool(name="apool", bufs=3) as apool,
                tc.tile_pool(name="opool", bufs=3) as opool,
                tc.tile_pool(name="ps3", bufs=4, space="PSUM") as ps3,
            ):
                for b in range(B):
                    for tt in range(S // D):
                        a_sb = apool.tile([D, NHT, D], BF16, tag="a_sb")
                        nc.sync.dma_start(
                            a_sb[:],
                            att_all[b][:, tt * D : (tt + 1) * D].rearrange(
                                "(co p) t -> p co t", p=D
                            ),
                        )
                        psum_o = ps3.tile([D, CH], F32, tag="ps_o")
                        for ct in range(NHT):
                            nc.tensor.matmul(
                                psum_o[:],
                                a_sb[:, ct, :],
                                wo_sb[:, ct, :],
                                start=(ct == 0),
                                stop=(ct == NHT - 1),
                            )
                        o_sb = opool.tile([D, CH], F32, tag="o_sb")
                        nc.vector.tensor_copy(o_sb[:], psum_o[:])
                        nc.sync.dma_start(
                            out[b * S + tt * D : b * S + (tt + 1) * D, :], o_sb[:]
                        )

    nc.compile()
    return nc, maskT is not None


def kernel(hidden_states, attention_mask, position_ids, W_pack, W_o):
    _ensure_trace_hook()
    hidden_states = np.asarray(hidden_states, dtype=np.float32)
    attention_mask = np.asarray(attention_mask, dtype=np.float32)
    position_ids = np.asarray(position_ids)
    W_pack = np.asarray(W_pack, dtype=np.float32)
    W_o = np.asarray(W_o, dtype=np.float32)

    ops, need_dma = _classify_mask(attention_mask)

    key = (ops.tobytes(), need_dma)
    if key not in _cache:
        _cache.clear()
        _cache[key] = _build(ops, need_dma)
    nc, has_mask_param = _cache[key]

    # ---- host-side prep ----
    X_T = np.ascontiguousarray(hidden_states.reshape(T, H).T).astype(ml_dtypes.bfloat16)

    # RoPE tables (position-gathered), transposed to [d, t]; scale folded into Q's.
    pos = position_ids.reshape(T).astype(np.float32)
    inv_freq = (1.0 / (BASE ** (np.arange(0, D, 2, dtype=np.float32) / D))).astype(
        np.float32
    )
    ang = pos[:, None] * inv_freq[None, :]          # [T, 64]
    ang = np.concatenate([ang, ang], axis=1)         # [T, 128]
    cos = np.cos(ang).astype(np.float32)
    sin = np.sin(ang).astype(np.float32)
    sin_signed = sin.copy()
    sin_signed[:, :64] *= -1.0                       # rows d<64 multiply -q[d+64]
    isd = np.float32(1.0 / math.sqrt(D))
    tabs = np.stack(
        [
            (cos * isd).T,
            (sin_signed * isd).T,
            cos.T,
            sin_signed.T,
        ]
    ).astype(np.float32)                             # [4, 128, T]
    tabs = np.ascontiguousarray(tabs)

    maskT_np = None
    if has_mask_param:
        maskT_np = np.ascontiguousarray(
            np.transpose(attention_mask[:, 0], (0, 2, 1))
        ).astype(np.float32)                         # [B, S(k), S(q)]

    in_maps = []
    for c in range(N_CORES):
        qr = slice(c * CH, (c + 1) * CH)
        kr = slice(H + c * CH, H + (c + 1) * CH)
        vr = slice(2 * H + c * CH, 2 * H + (c + 1) * CH)
        wqk_c = np.ascontiguousarray(
            np.concatenate([W_pack[qr], W_pack[kr]], axis=0).T
        ).astype(ml_dtypes.bfloat16)                 # [H, 1024]
        wv_c = np.ascontiguousarray(W_pack[vr].T).astype(ml_dtypes.bfloat16)  # [H, 512]
        wo_c = np.ascontiguousarray(W_o[c * CH : (c + 1) * CH, :].T).astype(
            ml_dtypes.bfloat16
        )                                            # [H, 512]
        m = {"x_t": X_T, "wqk": wqk_c, "wv": wv_c, "wo": wo_c, "tabs": tabs}
        if has_mask_param:
            m["maskT"] = maskT_np
        in_maps.append(m)

    import os

    trace = bool(os.environ.get("BASS_TRACE"))
    res = run_bass_kernel_spmd(
        nc, in_maps, core_ids=list(range(N_CORES)), trace=trace
    )
    last_run_info["exec_time_ns"] = res.exec_time_ns
    last_run_info["profile_json"] = getattr(res, "profile_json", None)

    outs = [res.results[c]["out"].reshape(B, S, CH) for c in range(N_CORES)]
    return np.concatenate(outs, axis=2)


# revision 9
# speedup vs baseline: 1.1379x; 1.0132x over previous
"""Distributed Bass kernel for nn_Attention (dense transformer attention block).

Tensor-parallel over heads across 8 TRN2 NeuronCores:
  - each core owns 4 heads: its slice of W_pack (QKV) and the matching
    512 input channels of o_proj,
  - attention (RoPE + causal softmax) is computed fully locally per head,
  - attention outputs are AllGathered (bf16) and each core computes a
    distinct 512-column slice of the o_proj output; the host concatenates
    the slices (no AllReduce needed).

Compute dtype: bf16 matmuls with f32 PSUM accumulation, f32 softmax.
Scores are computed transposed ([k, q] layout) so the softmax exp writes
P^T directly and PV needs no transpose; row sums come from a ones-vector
matmul and the normalization is applied to the (much smaller) attention
output via a K=1 broadcast matmul.
"""

import sys
import types
import math

sys.path.insert(0, "/opt/trn_rl_repo")

import numpy as np
import ml_dtypes

from concourse import bacc, tile, mybir
from concourse.bass_utils import run_bass_kernel_spmd

BF16 = mybir.dt.bfloat16
F32 = mybir.dt.float32

B = 2
S = 2048
H = 4096
NH = 32
D = 128
T = B * S
N_CORES = 8
HEADS_PER_CORE = NH // N_CORES          # 4
CH = HEADS_PER_CORE * D                 # 512 channels per core
BASE = 10000.0
NEG = np.float32(np.finfo(np.float32).min)

# mask-block ops (per [q-chunk=512, k-tile=128] block, scoresT layout)
SKIP, NOMASK, DIAG, DMAMASK = 0, 1, 2, 3

QC = S // 512                            # 4 q-chunks per batch
KT = S // 128                            # 16 k-tiles per batch

_cache = {}
last_run_info = {}


def _ensure_trace_hook():
    """Register the NTFF profile hook missing from this image's antenv."""
    if "antenv.axon_hooks" in sys.modules:
        return
    try:
        from trn_agent_boot.trn_boot import _ntff_profile_via_ctypes

        hook = _ntff_profile_via_ctypes("/opt/axon/libaxon_pjrt.so")
        mod = types.ModuleType("antenv.axon_hooks")
        mod.get_axon_ntff_profile_hook = lambda: hook
        mod.set_axon_ntff_profile_hook = lambda h: None
        sys.modules["antenv.axon_hooks"] = mod
        from concourse import bass_utils

        bass_utils.upload_artifacts = lambda tmpdir: tmpdir
    except Exception:
        pass


def _classify_mask(mask):
    """Per (b, q-chunk 512, k-tile 128) block op for the scoresT layout."""
    ops = np.empty((B, QC, KT), dtype=np.int32)
    karr = np.arange(128)
    qarr = np.arange(512)
    need_dma = False
    for b in range(B):
        mb = np.asarray(mask[b, 0])
        for qc in range(QC):
            qs = qc * 512
            for kt in range(KT):
                ks = kt * 128
                sub = mb[qs : qs + 512, ks : ks + 128]  # [q, k]
                if np.all(sub <= -1e30):
                    ops[b, qc, kt] = SKIP
                elif not sub.any():
                    ops[b, qc, kt] = NOMASK
                else:
                    delta = ks - qs
                    if 0 <= delta <= 384:
                        pat = np.where(
                            (delta + karr[None, :]) > qarr[:, None], NEG, np.float32(0)
                        )
                        if np.array_equal(sub, pat):
                            ops[b, qc, kt] = DIAG
                            continue
                    ops[b, qc, kt] = DMAMASK
                    need_dma = True
    return ops, need_dma


def _build(ops, need_dma):
    nc = bacc.Bacc(None, target_bir_lowering=False)

    x_t = nc.declare_dram_parameter("x_t", [H, T], BF16, isOutput=False)
    wqk = nc.declare_dram_parameter("wqk", [H, 2 * CH], BF16, isOutput=False)
    wv = nc.declare_dram_parameter("wv", [H, CH], BF16, isOutput=False)
    wo = nc.declare_dram_parameter("wo", [H, CH], BF16, isOutput=False)
    tabs = nc.declare_dram_parameter("tabs", [4, D, T], F32, isOutput=False)
    maskT = None
    if need_dma:
        maskT = nc.declare_dram_parameter("maskT", [B, S, S], F32, isOutput=False)
    out = nc.declare_dram_parameter("out", [T, CH], F32, isOutput=True)

    ones_col_np = np.ones((D, 1), dtype=ml_dtypes.bfloat16)
    ones_row_np = np.ones((1, D), dtype=np.float32)
    # maskfull[k, j] = NEG if k > j - 384 else 0   (slice [384-delta : 896-delta])
    j = np.arange(896)
    maskfull_np = np.where(np.arange(D)[:, None] > (j[None, :] - 384), NEG, 0.0).astype(
        np.float32
    )

    rg = [list(range(N_CORES))]
    NHT = H // D  # 32 h-tiles

    with tile.TileContext(nc) as tc:
        with (
            tc.tile_pool(name="dram", bufs=1, space="DRAM") as dram,
            tc.tile_pool(name="const", bufs=1) as constp,
            tc.tile_pool(name="wpool", bufs=1) as wpool,
        ):
            qt_d = dram.tile([CH, T], BF16, tag="qt_d")
            kt_d = dram.tile([CH, T], BF16, tag="kt_d")
            v_d = dram.tile([T, CH], BF16, tag="v_d")
            att_loc = [
                dram.tile([CH, S], BF16, tag=f"att_loc{b}", name=f"att_loc{b}")
                for b in range(B)
            ]
            att_all = [
                dram.tile(
                    [N_CORES * CH, S],
                    BF16,
                    addr_space="Shared",
                    tag=f"att_all{b}",
                    name=f"att_all{b}",
                )
                for b in range(B)
            ]

            ones_col = constp.tile([D, 1], BF16, tag="ones_col")
            nc.sync.dma_start(ones_col[:], nc.inline_tensor(ones_col_np, "ones_col_c")[:])
            ones_row = constp.tile([1, D], F32, tag="ones_row")
            nc.sync.dma_start(ones_row[:], nc.inline_tensor(ones_row_np, "ones_row_c")[:])
            maskfull = constp.tile([D, 896], F32, tag="maskfull")
            nc.sync.dma_start(maskfull[:], nc.inline_tensor(maskfull_np, "maskfull_c")[:])

            # ------------- stage 1: QKV projection + RoPE (single X pass) -------------
            wqk_sb = wpool.tile([D, NHT, 2 * CH], BF16, tag="wqk")
            nc.sync.dma_start(wqk_sb[:], wqk.rearrange("(ho p) c -> p ho c", p=D))
            wv_sb = wpool.tile([D, NHT, CH], BF16, tag="wv")
            nc.sync.dma_start(wv_sb[:], wv.rearrange("(ho p) c -> p ho c", p=D))

            with (
                tc.tile_pool(name="xpool", bufs=2) as xpool,
                tc.tile_pool(name="tpool", bufs=1) as tpool,
                tc.tile_pool(name="rpool", bufs=2) as rpool,
                tc.tile_pool(name="qkout", bufs=2) as qkout,
                tc.tile_pool(name="ps1", bufs=6, space="PSUM") as ps1,
            ):
                for tci in range(T // 512):
                    t0 = tci * 512
                    x_tile = xpool.tile([D, NHT, 512], BF16, tag="x")
                    nc.sync.dma_start(
                        x_tile[:],
                        x_t[:, t0 : t0 + 512].rearrange("(ho p) t -> p ho t", p=D),
                    )
                    tb = tpool.tile([D, 4, 512], F32, tag="tb")
                    for ti in range(4):
                        nc.sync.dma_start(tb[:, ti, :], tabs[ti, :, t0 : t0 + 512])

                    for ct in range(2 * CH // D):  # 0-3: q heads, 4-7: k heads
                        psum = ps1.tile([D, 512], F32, tag="ps1")
                        for h in range(NHT):
                            nc.tensor.matmul(
                                psum[:],
                                wqk_sb[:, h, ct * D : (ct + 1) * D],
                                x_tile[:, h, :],
                                start=(h == 0),
                                stop=(h == NHT - 1),
                            )
                        is_q = ct < HEADS_PER_CORE
                        cos_i = 0 if is_q else 2
                        sin_i = 1 if is_q else 3
                        tmp1 = rpool.tile([D, 512], F32, tag="tmp1")
                        nc.vector.tensor_mul(tmp1[:], psum[:], tb[:, cos_i, :])
                        tmp2 = rpool.tile([D, 512], F32, tag="tmp2")
                        nc.vector.tensor_mul(
                            tmp2[0:64, :], psum[64:128, :], tb[0:64, sin_i, :]
                        )
                        nc.vector.tensor_mul(
                            tmp2[64:128, :], psum[0:64, :], tb[64:128, sin_i, :]
                        )
                        qk_bf = qkout.tile([D, 512], BF16, tag="qk_bf")
                        nc.vector.tensor_add(qk_bf[:], tmp1[:], tmp2[:])
                        head = ct % HEADS_PER_CORE
                        dst = qt_d if is_q else kt_d
                        nc.sync.dma_start(
                            dst[head * D : (head + 1) * D, t0 : t0 + 512], qk_bf[:]
                        )

                    for ts in range(4):  # V: [t, ch] layout
                        psum = ps1.tile([D, 512], F32, tag="ps1", name="psum_v")
                        for h in range(NHT):
                            nc.tensor.matmul(
                                psum[:],
                                x_tile[:, h, ts * D : (ts + 1) * D],
                                wv_sb[:, h, :],
                                start=(h == 0),
                                stop=(h == NHT - 1),
                            )
                        v_bf = qkout.tile([D, CH], BF16, tag="v_bf")
                        nc.vector.tensor_copy(v_bf[:], psum[:])
                        nc.sync.dma_start(
                            v_d[t0 + ts * D : t0 + (ts + 1) * D, :], v_bf[:]
                        )

            # ------------- stage 2: attention per (batch, head) -------------
            with (
                tc.tile_pool(name="kqv", bufs=2) as kqv,
                tc.tile_pool(name="ppool", bufs=6) as ppool,
                tc.tile_pool(name="mpool", bufs=3) as mpool,
                tc.tile_pool(name="epi", bufs=3) as epi,
                tc.tile_pool(name="ps_s", bufs=4, space="PSUM") as ps_s,
                tc.tile_pool(name="ps_av", bufs=2, space="PSUM") as ps_av,
                tc.tile_pool(name="ps_sum", bufs=1, space="PSUM") as ps_sum,
                tc.tile_pool(name="ps_bc", bufs=1, space="PSUM") as ps_bc,
            ):
                pending = []  # deferred per-kt tails and epilogues

                def drain_pending(n=None):
                    todo = pending[:] if n is None else pending[:n]
                    del pending[: len(todo)]
                    for fn in todo:
                        fn()

                for b in range(B):
                    for head in range(HEADS_PER_CORE):
                        k_sb = kqv.tile([D, S], BF16, tag="k_sb")
                        nc.sync.dma_start(
                            k_sb[:],
                            kt_d[head * D : (head + 1) * D, b * S : (b + 1) * S],
                        )
                        q_sb = kqv.tile([D, S], BF16, tag="q_sb")
                        nc.sync.dma_start(
                            q_sb[:],
                            qt_d[head * D : (head + 1) * D, b * S : (b + 1) * S],
                        )
                        v_sb = kqv.tile([D, KT, D], BF16, tag="v_sb")
                        nc.sync.dma_start(
                            v_sb[:],
                            v_d[
                                b * S : (b + 1) * S, head * D : (head + 1) * D
                            ].rearrange("(o p) c -> p o c", p=D),
                        )
                        for qc in range(QC):
                            kts = sorted(
                                [kt for kt in range(KT) if ops[b, qc, kt] != SKIP],
                                key=lambda kt: 0 if ops[b, qc, kt] != NOMASK else 1,
                            )
                            psum_av = ps_av.tile([D, 512], F32, tag="av")
                            psum_sum = ps_sum.tile([1, 512], F32, tag="sm")
                            n_kt = len(kts)
                            for i, kt in enumerate(kts):
                                psum_s = ps_s.tile([D, 512], F32, tag="s")
                                nc.tensor.matmul(
                                    psum_s[:],
                                    k_sb[:, kt * D : (kt + 1) * D],
                                    q_sb[:, qc * 512 : (qc + 1) * 512],
                                    start=True,
                                    stop=True,
                                )
                                op = ops[b, qc, kt]
                                if op == DIAG:
                                    delta = kt * D - qc * 512
                                    nc.vector.tensor_add(
                                        psum_s[:],
                                        psum_s[:],
                                        maskfull[:, 384 - delta : 896 - delta],
                                    )
                                elif op == DMAMASK:
                                    mt = mpool.tile([D, 512], F32, tag="mt")
                                    nc.sync.dma_start(
                                        mt[:],
                                        maskT[
                                            b,
                                            kt * D : (kt + 1) * D,
                                            qc * 512 : (qc + 1) * 512,
                                        ],
                                    )
                                    nc.vector.tensor_add(psum_s[:], psum_s[:], mt[:])

                                def tail(
                                    psum_s=psum_s,
                                    psum_av=psum_av,
                                    psum_sum=psum_sum,
                                    i=i,
                                    kt=kt,
                                    n_kt=n_kt,
                                    v_sb=v_sb,
                                ):
                                    pexp = ppool.tile(
                                        [D, 512], BF16, tag="pexp", name="pexp"
                                    )
                                    nc.scalar.activation(
                                        pexp[:],
                                        psum_s[:],
                                        mybir.ActivationFunctionType.Exp,
                                    )
                                    nc.tensor.matmul(
                                        psum_sum[:],
                                        ones_col[:],
                                        pexp[:],
                                        start=(i == 0),
                                        stop=(i == n_kt - 1),
                                    )
                                    nc.tensor.matmul(
                                        psum_av[:],
                                        v_sb[:, kt, :],
                                        pexp[:],
                                        start=(i == 0),
                                        stop=(i == n_kt - 1),
                                    )

                                pending.append(tail)
                                # keep ~3 score-matmuls in flight ahead of the tails
                                if len(pending) > 3:
                                    drain_pending(len(pending) - 3)

                            def epilogue(
                                psum_av=psum_av,
                                psum_sum=psum_sum,
                                b=b,
                                head=head,
                                qc=qc,
                            ):
                                sums_sb = epi.tile([1, 512], F32, tag="sums_sb", name="sums_sb")
                                nc.scalar.copy(sums_sb[:], psum_sum[:])
                                psum_bc = ps_bc.tile([D, 512], F32, tag="bc", name="bc")
                                nc.tensor.matmul(
                                    psum_bc[:], ones_row[:], sums_sb[:], start=True, stop=True
                                )
                                bc_sb = epi.tile([D, 512], F32, tag="bc_sb", name="bc_sb")
                                nc.vector.reciprocal_approx_fast(bc_sb[:], psum_bc[:])
                                attn_sb = epi.tile(
                                    [D, 512], BF16, tag="attn_sb", name="attn_sb"
                                )
                                nc.vector.tensor_mul(attn_sb[:], psum_av[:], bc_sb[:])
                                nc.sync.dma_start(
                                    att_loc[b][
                                        head * D : (head + 1) * D,
                                        qc * 512 : (qc + 1) * 512,
                                    ],
                                    attn_sb[:],
                                )

                            pending.append(epilogue)

                    # all heads of batch b issued; flush and gather
                    drain_pending()
                    nc.gpsimd.collective_compute(
                        "AllGather",
                        mybir.AluOpType.bypass,
                        replica_groups=rg,
                        ins=[att_loc[b].opt()],
                        outs=[att_all[b].opt()],
                    )

            # ------------- stage 3: o_proj slice -------------
            wo_sb = wpool.tile([D, NHT, 2 * CH], BF16, tag="wqk", name="wo_sb")[:, :, :CH]
            nc.sync.dma_start(wo_sb[:], wo.rearrange("(co p) o -> p co o", p=D))
            with (
                tc.tile_pool(name="apool", bufs=3) as apool,
                tc.tile_pool(name="opool", bufs=3) as opool,
                tc.tile_pool(name="ps3", bufs=4, space="PSUM") as ps3,
            ):
                for b in range(B):
                    for tt in range(S // D):
                        a_sb = apool.tile([D, NHT, D], BF16, tag="a_sb")
                        nc.sync.dma_start(
                            a_sb[:],
                            att_all[b][:, tt * D : (tt + 1) * D].rearrange(
                                "(co p) t -> p co t", p=D
                            ),
                        )
                        psum_o = ps3.tile([D, CH], F32, tag="ps_o")
                        for ct in range(NHT):
                            nc.tensor.matmul(
                                psum_o[:],
                                a_sb[:, ct, :],
                                wo_sb[:, ct, :],
                                start=(ct == 0),
                                stop=(ct == NHT - 1),
                            )
                        o_sb = opool.tile([D, CH], F32, tag="o_sb")
                        nc.vector.tensor_copy(o_sb[:], psum_o[:])
                        nc.sync.dma_start(
                            out[b * S + tt * D : b * S + (tt + 1) * D, :], o_sb[:]
                        )

    nc.compile()
    return nc, maskT is not None


def kernel(hidden_states, attention_mask, position_ids, W_pack, W_o):
    _ensure_trace_hook()
    hidden_states = np.asarray(hidden_states, dtype=np.float32)
    attention_mask = np.asarray(attention_mask, dtype=np.float32)
    position_ids = np.asarray(position_ids)
    W_pack = np.asarray(W_pack, dtype=np.float32)
    W_o = np.asarray(W_o, dtype=np.float32)

    ops, need_dma = _classify_mask(attention_mask)

    key = (ops.tobytes(), need_dma)
    if key not in _cache:
        _cache.clear()
        _cache[key] = _build(ops, need_dma)
    nc, has_mask_param = _cache[key]

    # ---- host-side prep ----
    X_T = np.ascontiguousarray(hidden_states.reshape(T, H).T).astype(ml_dtypes.bfloat16)

    # RoPE tables (position-gathered), transposed to [d, t]; scale folded into Q's.
    pos = position_ids.reshape(T).astype(np.float32)
    inv_freq = (1.0 / (BASE ** (np.arange(0, D, 2, dtype=np.float32) / D))).astype(
        np.float32
    )
    ang = pos[:, None] * inv_freq[None, :]          # [T, 64]
    ang = np.concatenate([ang, ang], axis=1)         # [T, 128]
    cos = np.cos(ang).astype(np.float32)
    sin = np.sin(ang).astype(np.float32)
    sin_signed = sin.copy()
    sin_signed[:, :64] *= -1.0                       # rows d<64 multiply -q[d+64]
    isd = np.float32(1.0 / math.sqrt(D))
    tabs = np.stack(
        [
            (cos * isd).T,
            (sin_signed * isd).T,
            cos.T,
            sin_signed.T,
        ]
    ).astype(np.float32)                             # [4, 128, T]
    tabs = np.ascontiguousarray(tabs)

    maskT_np = None
    if has_mask_param:
        maskT_np = np.ascontiguousarray(
            np.transpose(attention_mask[:, 0], (0, 2, 1))
        ).astype(np.float32)                         # [B, S(k), S(q)]

    in_maps = []
    for c in range(N_CORES):
        qr = slice(c * CH, (c + 1) * CH)
        kr = slice(H + c * CH, H + (c + 1) * CH)
        vr = slice(2 * H + c * CH, 2 * H + (c + 1) * CH)
        wqk_c = np.ascontiguousarray(
            np.concatenate([W_pack[qr], W_pack[kr]], axis=0).T
        ).astype(ml_dtypes.bfloat16)                 # [H, 1024]
        wv_c = np.ascontiguousarray(W_pack[vr].T).astype(ml_dtypes.bfloat16)  # [H, 512]
        wo_c = np.ascontiguousarray(W_o[c * CH : (c + 1) * CH, :].T).astype(
            ml_dtypes.bfloat16
        )                                            # [H, 512]
        m = {"x_t": X_T, "wqk": wqk_c, "wv": wv_c, "wo": wo_c, "tabs": tabs}
        if has_mask_param:
            m["maskT"] = maskT_np
        in_maps.append(m)

    import os

    trace = bool(os.environ.get("BASS_TRACE"))
    res = run_bass_kernel_spmd(
        nc, in_maps, core_ids=list(range(N_CORES)), trace=trace
    )
    last_run_info["exec_time_ns"] = res.exec_time_ns
    last_run_info["profile_json"] = getattr(res, "profile_json", None)

    outs = [res.results[c]["out"].reshape(B, S, CH) for c in range(N_CORES)]
    return np.concatenate(outs, axis=2)
